# revision 52
# baseline (speedup 1.0000x reference)
"""Expert-parallel grouped matmul (MoE BatchLinear) for 8 Trainium2 NeuronCores.

Problem: y[t] = x[t] @ W[g(t)] where tokens are grouped contiguously by expert
g (G=64 experts, counts given at runtime). Sharding: expert-parallel — core c
owns experts [8c, 8c+8) and the contiguous token rows routed to them. The
"all-to-all" is done host-side: kernel() receives full inputs, slices/pads
per-core token blocks, and scatters per-core outputs back.

Device kernel (SPMD, one program on 8 cores):
  ~17 warmup matmuls on a zeroed tile run during the initial DMA wait,
  releasing the PE HAM clock-gate (1.2->2.4 GHz needs ~3.4us of activity)
  so the real stream starts warm.
  for each local expert e (pairwise big-first order: a big expert's long
  compute window lets the W prefetch run ahead of the next small expert's
  burst demand):
    xT_e resident in SBUF as [128ki, 8ko, Te] (host pre-transposed); the
    FIRST expert's xT instead ships as per-m-tile 256KB chunks, FIFO-
    interleaved with its W ladder so the first matmul gates on only
    ~0.75MB of DMA (the Sync HWDGE ring is FIFO: time-to-data is
    cumulative-bytes-ahead / wire rate)
    for each NQW-wide n-slab of W_e (slab [128ki, 8ko, NQW], 4 bufs;
    first expert laddered 256/256/512/1024/2048, all its DMAs pre-emitted
    before its compute loops so consumers see the right dependencies):
      for each PAIR of 128-token m-tiles (one staging tile + one y DMA per
      pair halves y-DMA count; small y DMAs have ~2.5us fixed completion):
        8 k-steps x NB matmuls (N=512, or one N<=512 on the ladder head)
        accumulate into NB full-bank PSUM tiles (sub-bank tiles are unsafe:
        start=True clears has_written for the whole physical bank)
        PSUM -> fp16 SBUF staging, copies split between the Vector and
        Scalar engines (even/odd banks), DMA staging -> y (scalar ring;
        narrow slabs use a deeper staging pool so their y-write completion
        latency never throttles the PE)

All DRAM layouts are chosen so every DMA reads/writes fully-contiguous
per-partition runs: W as [e, q, ki, ko, n], xT as per-expert [ki, ko, Te]
blocks (first expert: per-m [ki, ko, 128] chunks), y as [q, 128, mtile,
NQW] fp16 (reordered/upcast host-side).

Numerics: operands stream as fp16 (1 PE cycle/row, fp32 PSUM accumulation),
y stored fp16. Measured absmax/scale error ~4.8e-4 vs the fp32 reference.

Measured on 8 axon trn2 cores: 464.2us HW exec (baseline 511.3us); per-core
MM stream runs at the warm floor (442us busy, zero gaps); remaining
overhead is ~7us fixed preamble, ~7.5us head DMA wait (covered by warmups),
~13us fixed tail (final-DMA receipt + teardown barrier). Runs occasionally
hit a chip-wide P0 downclock (PE 2.4->2.0GHz, every MM exactly 259ns,
+20% wall; detect via MM median duration).
"""

import numpy as np

G, N_TOK, D_IN, D_OUT, CAP = 64, 32768, 1024, 4096, 768
M_CORES = 8
EPC = G // M_CORES          # experts per core
P = 128                     # partitions / k-tile / m-tile
KO = D_IN // P              # 8 k-tiles
MODE = "f16"                # "f16" | "f32r" | "f32"
NQW = 2048 if MODE == "f16" else 1024   # n-slab width (SBUF budget bound)
NQ = D_OUT // NQW
NB = NQW // 512             # psum banks per slab

_cache = {}


def _mm_dt(mybir):
    return {
        "f16": mybir.dt.float16,
        "f32r": mybir.dt.float32r,
        "f32": mybir.dt.float32,
    }[MODE]


def _np_dt():
    return np.float16 if MODE == "f16" else np.float32




def _slot_order(mt):
    """Pairwise big-first order: within each adjacent slot pair process the
    bigger expert first. A big expert's long compute window (many m-tiles
    per W byte) lets the W prefetch stream run ahead, so the following
    small expert's burst demand is already buffered."""
    order = []
    for p in range(0, EPC - 1, 2):
        order += sorted((p, p + 1), key=lambda j: -mt[j])
    if EPC % 2:
        order.append(EPC - 1)
    return [j for j in order if mt[j] > 0]

def _build(mt):
    """Compile the SPMD program for per-expert-slot m-tile counts mt (len EPC)."""
    import concourse.mybir as mybir
    import concourse.tile as tile
    from concourse import bacc

    f32 = mybir.dt.float32
    f16 = mybir.dt.float16
    fmm = _mm_dt(mybir)
    n_mtiles = sum(mt)

    nc = bacc.Bacc("TRN2", target_bir_lowering=False, debug=False)
    order = _slot_order(mt)
    e_first = order[0] if order else -1
    xt_d = {
        e: nc.dram_tensor(f"xT{e}", [P, KO, P * mt[e]], fmm, kind="ExternalInput")
        for e in range(EPC)
        if mt[e] > 0 and e != e_first
    }
    # first expert's xT ships as per-m-tile 256KB chunks: the Sync ring is
    # FIFO, so the first matmul gates on xa0+slab0 = 1.25MB instead of the
    # whole 1.5MB xT + 1MB slab
    xa_d = [
        nc.dram_tensor(f"xTa{m}", [P, KO, P], fmm, kind="ExternalInput")
        for m in range(mt[e_first] if order else 0)
    ]
    w_d = nc.dram_tensor("W", [EPC, NQ, P, KO, NQW], fmm, kind="ExternalInput")
    y_d = nc.dram_tensor("y", [NQ, P, n_mtiles, NQW], f16, kind="ExternalOutput")
    w_ap, y = w_d.ap(), y_d.ap()

    with tile.TileContext(nc) as tc:
        with (
            tc.tile_pool(name="wq", bufs=4) as wq_pool,
            tc.tile_pool(name="xt", bufs=2) as xt_pool,
            tc.tile_pool(name="st", bufs=2) as st_pool,
            # narrow (laddered) slabs emit small y-writes every ~1.7us, but
            # each small DMA has ~2.5us completion latency (HBM write
            # receipt); a deeper pool keeps the PE from throttling to the
            # y-completion rate
            tc.tile_pool(name="stn", bufs=6) as stn_pool,
            tc.tile_pool(name="xa", bufs=max(1, len(xa_d))) as xa_pool,
            tc.tile_pool(name="wz", bufs=1) as wz_pool,
            tc.tile_pool(name="ps", bufs=8, space="PSUM") as ps_pool,
        ):
            # ~42 warmup matmuls on a zeroed tile: they run during the
            # initial W/xT DMA wait, releasing the PE HAM clock-gate
            # (1.2 -> 2.4 GHz takes ~3.4us of sustained PE activity) so the
            # real MM stream starts warm.
            wz = wz_pool.tile([P, 512], fmm, tag="wz", name="wz")
            nc.vector.memset(wz[:], 0)
            psw = ps_pool.tile([P, 512], f32, tag="ps", name="psw")
            for _ in range(17):
                nc.tensor.matmul(psw[:], wz[:, 0:P], wz[:], start=True, stop=True)

            xa = [
                xa_pool.tile([P, KO, P], fmm, tag="xa", name="xa")
                for _ in range(len(xa_d))
            ]
            mi0 = 0  # global m-tile index
            for ei, e in enumerate(order):
                te = P * mt[e]
                if ei == 0:
                    nc.sync.dma_start(out=xa[0][:], in_=xa_d[0].ap())
                    xt = None
                else:
                    xt = xt_pool.tile([P, KO, te], fmm, tag="xt")
                    nc.sync.dma_start(out=xt[:], in_=xt_d[e].ap())
                # ladder-size the first expert's slabs so the first matmul
                # only waits on a 256-wide W transfer
                if NQW >= 2048 and ei == 0:
                    widths = [256, 256, 512, 1024]
                    widths += [NQW] * ((D_OUT - sum(widths)) // NQW)
                else:
                    widths = [NQW] * NQ
                wq_tiles = {}
                if ei == 0:
                    # pre-emit ALL of this expert's DMAs, xT chunks FIFO-
                    # interleaved between the early W slabs so arrivals pace
                    # just ahead of consumption. Emission must precede the
                    # compute loops in program order or the consumers see no
                    # dependency on the later chunks.
                    c2 = 0
                    for wi, wd in enumerate(widths):
                        q2, ncol2 = c2 // NQW, c2 % NQW
                        wq = wq_pool.tile([P, KO, wd], fmm, tag="wq", name="wq")
                        nc.sync.dma_start(
                            out=wq[:], in_=w_ap[e, q2, :, :, ncol2 : ncol2 + wd]
                        )
                        wq_tiles[wi] = wq
                        if wi == 0:
                            for mc in (1, 2, 3):
                                if mc < len(xa):
                                    nc.sync.dma_start(
                                        out=xa[mc][:], in_=xa_d[mc].ap()
                                    )
                        elif wi == 1:
                            for mc in range(4, len(xa)):
                                nc.sync.dma_start(
                                    out=xa[mc][:], in_=xa_d[mc].ap()
                                )
                        c2 += wd
                col = 0
                for wi, wd in enumerate(widths):
                    q, ncol, nb = col // NQW, col % NQW, max(1, wd // 512)
                    bw = min(512, wd)  # bank width (<512 only on ladder head)
                    if ei == 0:
                        wq = wq_tiles[wi]
                    else:
                        wq = wq_pool.tile([P, KO, wd], fmm, tag="wq", name="wq")
                        nc.sync.dma_start(
                            out=wq[:], in_=w_ap[e, q, :, :, ncol : ncol + wd]
                        )
                    get_w = lambda k, nn, wq=wq, bw=bw: wq[
                        :, k, nn * 512 : nn * 512 + bw
                    ]
                    m = 0
                    while m < mt[e]:
                        # batch two m-tiles per staging tile / y DMA: halves
                        # the y-DMA (and semaphore) count, whose fixed
                        # ~2.5us completion latency otherwise dominates
                        # narrow slabs and stretches the teardown epilogue
                        mb = min(2, mt[e] - m)
                        pool = st_pool if wd >= NQW else stn_pool
                        st = pool.tile([P, mb, wd], f16, tag="st", name="st")
                        for mh in range(mb):
                            # psum tiles are ALWAYS a full bank: sub-bank
                            # tiles can share a physical bank, and start=True
                            # clears has_written for the whole bank, which
                            # corrupts a co-resident accumulation group
                            pss = [
                                ps_pool.tile([P, 512], f32, tag="ps", name="ps")
                                for _ in range(nb)
                            ]
                            for k in range(KO):
                                if ei == 0:
                                    lhsT = xa[m + mh][:, k, :]
                                else:
                                    lhsT = xt[:, k, (m + mh) * P : (m + mh + 1) * P]
                                for nn in range(nb):
                                    nc.tensor.matmul(
                                        pss[nn][:, 0:bw],
                                        lhsT,
                                        get_w(k, nn),
                                        start=(k == 0),
                                        stop=(k == KO - 1),
                                    )
                            for nn in range(nb):
                                dst = st[:, mh, nn * 512 : nn * 512 + bw]
                                if nn % 2 == 0:
                                    nc.vector.tensor_copy(dst, pss[nn][:, 0:bw])
                                else:
                                    nc.scalar.copy(dst, pss[nn][:, 0:bw])
                        last_dma = (
                            ei == len(order) - 1
                            and wi == len(widths) - 1
                            and m + mb >= mt[e]
                        )
                        # final y write rides the (by now idle) Sync ring:
                        # tests whether the ~10us trigger->completion latency
                        # of the last DMA is a busy-ring artifact
                        dma_eng = nc.sync if last_dma else nc.scalar
                        dma_eng.dma_start(
                            out=y[q, :, mi0 + m : mi0 + m + mb, ncol : ncol + wd],
                            in_=st[:],
                        )
                        m += mb
                    col += wd
                mi0 += mt[e]
    nc.compile()
    return nc


def _prepare(x, weight, counts):
    """Host-side all-to-all: per-core padded token blocks + weight slices."""
    ndt = _np_dt()
    starts = np.zeros(G + 1, np.int64)
    np.cumsum(counts, out=starts[1:])
    cnt = counts.reshape(M_CORES, EPC)
    mt = tuple(int(v) for v in np.ceil(cnt / P).astype(np.int64).max(axis=0))

    order = _slot_order(mt)
    e_first = order[0] if order else -1
    in_maps, metas = [], []
    for c in range(M_CORES):
        im = {}
        meta = []
        mi0 = 0
        mi0_by_slot = {}
        for j in order:
            mi0_by_slot[j] = mi0
            mi0 += mt[j]
        for j in range(EPC):
            g = c * EPC + j
            s, n = int(starts[g]), int(counts[g])
            n = min(n, N_TOK - s) if s < N_TOK else 0
            if mt[j] == 0:
                continue
            te = P * mt[j]
            xe = np.zeros((te, D_IN), ndt)
            if n > 0:
                xe[:n] = x[s : s + n]
            # [te, D_IN] -> [D_IN, te] -> [KO, P, te] -> [P, KO, te]
            xT = np.ascontiguousarray(xe.T.reshape(KO, P, te).transpose(1, 0, 2))
            if j == e_first:
                # first-processed expert ships as per-m-tile contiguous chunks
                for m in range(mt[j]):
                    im[f"xTa{m}"] = np.ascontiguousarray(
                        xT[:, :, m * P : (m + 1) * P]
                    )
            else:
                im[f"xT{j}"] = xT
            meta.append((mi0_by_slot[j], s, n))
        # weight [EPC, D_IN, D_OUT] -> [e, q, ki, ko, n]
        wc = weight[c * EPC : (c + 1) * EPC].reshape(EPC, KO, P, NQ, NQW)
        im["W"] = np.ascontiguousarray(wc.transpose(0, 3, 2, 1, 4).astype(ndt))
        in_maps.append(im)
        metas.append(meta)
    return mt, in_maps, metas


def _ensure_axon_hooks_shim():
    """bass_utils imports antenv.axon_hooks when tracing is requested (e.g.
    via a BASS_TRACE env var); some images lack that module. Install a no-op
    shim so the run degrades to untraced instead of crashing."""
    try:
        from antenv.axon_hooks import get_axon_ntff_profile_hook  # noqa: F401
        return
    except ImportError:
        pass
    import sys
    import types

    try:
        import antenv
    except ImportError:
        return
    mod = types.ModuleType("antenv.axon_hooks")
    mod._hook = None
    mod.get_axon_ntff_profile_hook = lambda: getattr(mod, "_hook", None)

    def _set(h):
        mod._hook = h

    mod.set_axon_ntff_profile_hook = _set
    sys.modules["antenv.axon_hooks"] = mod
    antenv.axon_hooks = mod


def _run(x, weight, counts, trace=False, trace_cores=None):
    from concourse.bass_utils import run_bass_kernel_spmd

    _ensure_axon_hooks_shim()

    x = np.ascontiguousarray(np.asarray(x, dtype=np.float32))
    weight = np.ascontiguousarray(np.asarray(weight, dtype=np.float32))
    counts = np.asarray(counts).astype(np.int64)
    assert counts.shape == (G,)

    mt, in_maps, metas = _prepare(x, weight, counts)
    if sum(mt) == 0:
        return np.zeros((N_TOK, D_OUT), np.float32), None
    if mt not in _cache:
        _cache[mt] = _build(mt)
    nc = _cache[mt]

    res = run_bass_kernel_spmd(
        nc,
        in_maps,
        core_ids=list(range(M_CORES)),
        trace=trace,
        trace_cores=trace_cores,
    )
    out = np.zeros((N_TOK, D_OUT), np.float32)
    for c in range(M_CORES):
        yc = np.asarray(res.results[c]["y"], dtype=np.float32)  # [NQ, P, n_mtiles, NQW]
        n_mtiles = yc.shape[2]
        # -> [n_mtiles, P, NQ, NQW] -> [n_mtiles*P, D_OUT]
        yc = yc.transpose(2, 1, 0, 3).reshape(n_mtiles * P, D_OUT)
        for mi0, s, n in metas[c]:
            if n > 0:
                out[s : s + n] = yc[mi0 * P : mi0 * P + n]
    return out, res


def kernel(x, weight, num_inputs_per_group):
    out, _ = _run(x, weight, num_inputs_per_group)
    return out



# revision 60
# speedup vs baseline: 1.2069x; 1.2069x over previous
"""Expert-parallel grouped matmul (MoE BatchLinear) for 8 Trainium2 NeuronCores.

Problem: y[t] = x[t] @ W[g(t)] where tokens are grouped contiguously by expert
g (G=64 experts, counts given at runtime). Sharding: expert-parallel — core c
owns experts [8c, 8c+8) and the contiguous token rows routed to them. The
"all-to-all" is done host-side: kernel() receives full inputs, slices/pads
per-core token blocks, and scatters per-core outputs back.

Device kernel (SPMD, one program on 8 cores):
  ~17 warmup matmuls on a zeroed tile run during the initial DMA wait,
  releasing the PE HAM clock-gate (1.2->2.4 GHz needs ~3.4us of activity)
  so the real stream starts warm.
  for each local expert e (pairwise big-first order: a big expert's long
  compute window lets the W prefetch run ahead of the next small expert's
  burst demand):
    xT_e resident in SBUF as [128ki, 8ko, Te] (host pre-transposed); the
    FIRST expert's xT instead ships as per-m-tile 256KB chunks, FIFO-
    interleaved with its W ladder so the first matmul gates on only
    ~0.75MB of DMA (the Sync HWDGE ring is FIFO: time-to-data is
    cumulative-bytes-ahead / wire rate)
    for each NQW-wide n-slab of W_e (slab [128ki, 8ko, NQW], 4 bufs;
    first expert laddered 256/256/512/1024/2048, all its DMAs pre-emitted
    before its compute loops so consumers see the right dependencies):
      for each PAIR of 128-token m-tiles (one staging tile + one y DMA per
      pair halves y-DMA count; small y DMAs have ~2.5us fixed completion):
        8 k-steps x NB matmuls (N=512, or one N<=512 on the ladder head)
        accumulate into NB full-bank PSUM tiles (sub-bank tiles are unsafe:
        start=True clears has_written for the whole physical bank)
        PSUM -> fp16 SBUF staging, copies split between the Vector and
        Scalar engines (even/odd banks), DMA staging -> y (scalar ring;
        narrow slabs use a deeper staging pool so their y-write completion
        latency never throttles the PE)

All DRAM layouts are chosen so every DMA reads/writes fully-contiguous
per-partition runs: W as [e, q, ki, ko, n], xT as per-expert [ki, ko, Te]
blocks (first expert: per-m [ki, ko, 128] chunks), y as [q, 128, mtile,
NQW] fp16 (reordered/upcast host-side).

Numerics: operands stream as fp16 (1 PE cycle/row, fp32 PSUM accumulation),
y stored fp16. Measured absmax/scale error ~4.8e-4 vs the fp32 reference.

Measured on 8 axon trn2 cores: 464.2us HW exec (baseline 511.3us); per-core
MM stream runs at the warm floor (442us busy, zero gaps); remaining
overhead is ~7us fixed preamble, ~7.5us head DMA wait (covered by warmups),
~13us fixed tail (final-DMA receipt + teardown barrier). Runs occasionally
hit a chip-wide P0 downclock (PE 2.4->2.0GHz, every MM exactly 259ns,
+20% wall; detect via MM median duration).
"""

import numpy as np

G, N_TOK, D_IN, D_OUT, CAP = 64, 32768, 1024, 4096, 768
M_CORES = 8
EPC = G // M_CORES          # experts per core
P = 128                     # partitions / k-tile / m-tile
KO = D_IN // P              # 8 k-tiles
MODE = "f16"                # "f16" | "f32r" | "f32"
NQW = 2048 if MODE == "f16" else 1024   # n-slab width (SBUF budget bound)
NQ = D_OUT // NQW
NB = NQW // 512             # psum banks per slab

_cache = {}


def _mm_dt(mybir):
    return {
        "f16": mybir.dt.float16,
        "f32r": mybir.dt.float32r,
        "f32": mybir.dt.float32,
    }[MODE]


def _np_dt():
    return np.float16 if MODE == "f16" else np.float32




def _slot_order(mt):
    """Pairwise big-first order: within each adjacent slot pair process the
    bigger expert first. A big expert's long compute window (many m-tiles
    per W byte) lets the W prefetch stream run ahead, so the following
    small expert's burst demand is already buffered."""
    order = []
    for p in range(0, EPC - 1, 2):
        order += sorted((p, p + 1), key=lambda j: -mt[j])
    if EPC % 2:
        order.append(EPC - 1)
    return [j for j in order if mt[j] > 0]

def _build(mt):
    """Compile the SPMD program for per-expert-slot m-tile counts mt (len EPC)."""
    import concourse.mybir as mybir
    import concourse.tile as tile
    from concourse import bacc

    f32 = mybir.dt.float32
    f16 = mybir.dt.float16
    fmm = _mm_dt(mybir)
    n_mtiles = sum(mt)

    nc = bacc.Bacc("TRN2", target_bir_lowering=False, debug=False)
    order = _slot_order(mt)
    e_first = order[0] if order else -1
    xt_d = {
        e: nc.dram_tensor(f"xT{e}", [P, KO, P * mt[e]], fmm, kind="ExternalInput")
        for e in range(EPC)
        if mt[e] > 0 and e != e_first
    }
    # first expert's xT ships as per-m-tile 256KB chunks (Sync ring is FIFO:
    # the first matmul gates on cumulative bytes ahead). The m=0 chunk and
    # the first 256-wide W slab are packed into ONE host-built blob
    # [P, KO, 128+256] so a single DMA (one trigger + one receipt) gates
    # the first matmul instead of two.
    hb_d = nc.dram_tensor("HB", [P, KO, P + 256], fmm, kind="ExternalInput")
    xa_d = [
        nc.dram_tensor(f"xTa{m}", [P, KO, P], fmm, kind="ExternalInput")
        for m in range(1, mt[e_first] if order else 1)
    ]
    w_d = nc.dram_tensor("W", [EPC, NQ, P, KO, NQW], fmm, kind="ExternalInput")
    y_d = nc.dram_tensor("y", [NQ, P, n_mtiles, NQW], f16, kind="ExternalOutput")
    w_ap, y = w_d.ap(), y_d.ap()

    with tile.TileContext(nc) as tc:
        with (
            tc.tile_pool(name="wq", bufs=4) as wq_pool,
            tc.tile_pool(name="xt", bufs=2) as xt_pool,
            tc.tile_pool(name="st", bufs=2) as st_pool,
            # narrow (laddered) slabs emit small y-writes every ~1.7us, but
            # each small DMA has ~2.5us completion latency (HBM write
            # receipt); a deeper pool keeps the PE from throttling to the
            # y-completion rate
            tc.tile_pool(name="stn", bufs=5) as stn_pool,
            tc.tile_pool(name="xa", bufs=max(1, len(xa_d))) as xa_pool,
            tc.tile_pool(name="hb", bufs=1) as hb_pool,
            tc.tile_pool(name="wz", bufs=1) as wz_pool,
            tc.tile_pool(name="ps", bufs=8, space="PSUM") as ps_pool,
        ):
            # ~42 warmup matmuls on a zeroed tile: they run during the
            # initial W/xT DMA wait, releasing the PE HAM clock-gate
            # (1.2 -> 2.4 GHz takes ~3.4us of sustained PE activity) so the
            # real MM stream starts warm.
            wz = wz_pool.tile([P, 512], fmm, tag="wz", name="wz")
            nc.vector.memset(wz[:], 0)
            psw = ps_pool.tile([P, 512], f32, tag="ps", name="psw")
            for _ in range(17):
                nc.tensor.matmul(psw[:], wz[:, 0:P], wz[:], start=True, stop=True)

            xa = [
                xa_pool.tile([P, KO, P], fmm, tag="xa", name="xa")
                for _ in range(len(xa_d))
            ]
            mi0 = 0  # global m-tile index
            for ei, e in enumerate(order):
                te = P * mt[e]
                if ei == 0:
                    hb = hb_pool.tile([P, KO, P + 256], fmm, tag="hb", name="hb")
                    nc.sync.dma_start(out=hb[:], in_=hb_d.ap())
                    for mc in range(len(xa)):
                        nc.sync.dma_start(out=xa[mc][:], in_=xa_d[mc].ap())
                    xt = None
                else:
                    xt = xt_pool.tile([P, KO, te], fmm, tag="xt")
                    nc.sync.dma_start(out=xt[:], in_=xt_d[e].ap())
                # ladder-size the first expert's slabs so the first matmul
                # only waits on a 256-wide W transfer
                if NQW >= 2048 and ei == 0:
                    widths = [256, 256, 512, 1024]
                    widths += [NQW] * ((D_OUT - sum(widths)) // NQW)
                else:
                    widths = [NQW] * NQ
                wq_tiles = {}
                if ei == 0:
                    # pre-emit ALL of this expert's DMAs, xT chunks FIFO-
                    # interleaved between the early W slabs so arrivals pace
                    # just ahead of consumption. Emission must precede the
                    # compute loops in program order or the consumers see no
                    # dependency on the later chunks.
                    c2 = 0
                    for wi, wd in enumerate(widths):
                        q2, ncol2 = c2 // NQW, c2 % NQW
                        if wi > 0:
                            # wi==0's W lives in the hb blob (no separate DMA)
                            wq = wq_pool.tile(
                                [P, KO, wd], fmm, tag="wq", name="wq"
                            )
                            nc.sync.dma_start(
                                out=wq[:],
                                in_=w_ap[e, q2, :, :, ncol2 : ncol2 + wd],
                            )
                            wq_tiles[wi] = wq
                        c2 += wd
                col = 0
                for wi, wd in enumerate(widths):
                    q, ncol, nb = col // NQW, col % NQW, max(1, wd // 512)
                    bw = min(512, wd)  # bank width (<512 only on ladder head)
                    if ei == 0 and wi == 0:
                        get_w = lambda k, nn, hb=hb: hb[:, k, P : P + 256]
                    else:
                        if ei == 0:
                            wq = wq_tiles[wi]
                        else:
                            wq = wq_pool.tile(
                                [P, KO, wd], fmm, tag="wq", name="wq"
                            )
                            nc.sync.dma_start(
                                out=wq[:], in_=w_ap[e, q, :, :, ncol : ncol + wd]
                            )
                        get_w = lambda k, nn, wq=wq, bw=bw: wq[
                            :, k, nn * 512 : nn * 512 + bw
                        ]
                    m = 0
                    while m < mt[e]:
                        # batch two m-tiles per staging tile / y DMA: halves
                        # the y-DMA (and semaphore) count, whose fixed
                        # ~2.5us completion latency otherwise dominates
                        # narrow slabs and stretches the teardown epilogue
                        mb = min(2, mt[e] - m)
                        pool = st_pool if wd >= NQW else stn_pool
                        st = pool.tile([P, mb, wd], f16, tag="st", name="st")
                        for mh in range(mb):
                            # psum tiles are ALWAYS a full bank: sub-bank
                            # tiles can share a physical bank, and start=True
                            # clears has_written for the whole bank, which
                            # corrupts a co-resident accumulation group
                            pss = [
                                ps_pool.tile([P, 512], f32, tag="ps", name="ps")
                                for _ in range(nb)
                            ]
                            for k in range(KO):
                                if ei == 0:
                                    if m + mh == 0:
                                        lhsT = hb[:, k, 0:P]
                                    else:
                                        lhsT = xa[m + mh - 1][:, k, :]
                                else:
                                    lhsT = xt[:, k, (m + mh) * P : (m + mh + 1) * P]
                                for nn in range(nb):
                                    nc.tensor.matmul(
                                        pss[nn][:, 0:bw],
                                        lhsT,
                                        get_w(k, nn),
                                        start=(k == 0),
                                        stop=(k == KO - 1),
                                    )
                            for nn in range(nb):
                                dst = st[:, mh, nn * 512 : nn * 512 + bw]
                                if nn % 2 == 0:
                                    nc.vector.tensor_copy(dst, pss[nn][:, 0:bw])
                                else:
                                    nc.scalar.copy(dst, pss[nn][:, 0:bw])
                        last_dma = (
                            ei == len(order) - 1
                            and wi == len(widths) - 1
                            and m + mb >= mt[e]
                        )
                        # final y write rides the (by now idle) Sync ring:
                        # tests whether the ~10us trigger->completion latency
                        # of the last DMA is a busy-ring artifact
                        dma_eng = nc.sync if last_dma else nc.scalar
                        dma_eng.dma_start(
                            out=y[q, :, mi0 + m : mi0 + m + mb, ncol : ncol + wd],
                            in_=st[:],
                        )
                        m += mb
                    col += wd
                mi0 += mt[e]
    nc.compile()
    return nc


def _prepare(x, weight, counts):
    """Host-side all-to-all: per-core padded token blocks + weight slices."""
    ndt = _np_dt()
    starts = np.zeros(G + 1, np.int64)
    np.cumsum(counts, out=starts[1:])
    cnt = counts.reshape(M_CORES, EPC)
    mt = tuple(int(v) for v in np.ceil(cnt / P).astype(np.int64).max(axis=0))

    order = _slot_order(mt)
    e_first = order[0] if order else -1
    in_maps, metas = [], []
    for c in range(M_CORES):
        im = {}
        meta = []
        mi0 = 0
        mi0_by_slot = {}
        for j in order:
            mi0_by_slot[j] = mi0
            mi0 += mt[j]
        for j in range(EPC):
            g = c * EPC + j
            s, n = int(starts[g]), int(counts[g])
            n = min(n, N_TOK - s) if s < N_TOK else 0
            if mt[j] == 0:
                continue
            te = P * mt[j]
            xe = np.zeros((te, D_IN), ndt)
            if n > 0:
                xe[:n] = x[s : s + n]
            # [te, D_IN] -> [D_IN, te] -> [KO, P, te] -> [P, KO, te]
            xT = np.ascontiguousarray(xe.T.reshape(KO, P, te).transpose(1, 0, 2))
            if j == e_first:
                # first-processed expert ships as per-m-tile contiguous
                # chunks; m=0's chunk is packed with the first 256-wide W
                # slab into the HB blob (added after im["W"] is built below)
                for m in range(1, mt[j]):
                    im[f"xTa{m}"] = np.ascontiguousarray(
                        xT[:, :, m * P : (m + 1) * P]
                    )
                im["_xa0"] = np.ascontiguousarray(xT[:, :, 0:P])
            else:
                im[f"xT{j}"] = xT
            meta.append((mi0_by_slot[j], s, n))
        # weight [EPC, D_IN, D_OUT] -> [e, q, ki, ko, n]
        wc = weight[c * EPC : (c + 1) * EPC].reshape(EPC, KO, P, NQ, NQW)
        im["W"] = np.ascontiguousarray(wc.transpose(0, 3, 2, 1, 4).astype(ndt))
        if "_xa0" in im:
            # HB blob: [P, KO, 128 (m=0 xT chunk) | 256 (first W slab cols)]
            im["HB"] = np.ascontiguousarray(
                np.concatenate(
                    [im.pop("_xa0"), im["W"][e_first, 0, :, :, 0:256]], axis=2
                )
            )
        in_maps.append(im)
        metas.append(meta)
    return mt, in_maps, metas


def _ensure_axon_hooks_shim():
    """bass_utils imports antenv.axon_hooks when tracing is requested (e.g.
    via a BASS_TRACE env var); some images lack that module. Install a no-op
    shim so the run degrades to untraced instead of crashing."""
    try:
        from antenv.axon_hooks import get_axon_ntff_profile_hook  # noqa: F401
        return
    except ImportError:
        pass
    import sys
    import types

    try:
        import antenv
    except ImportError:
        return
    mod = types.ModuleType("antenv.axon_hooks")
    mod._hook = None
    mod.get_axon_ntff_profile_hook = lambda: getattr(mod, "_hook", None)

    def _set(h):
        mod._hook = h

    mod.set_axon_ntff_profile_hook = _set
    sys.modules["antenv.axon_hooks"] = mod
    antenv.axon_hooks = mod


def _run(x, weight, counts, trace=False, trace_cores=None):
    from concourse.bass_utils import run_bass_kernel_spmd

    _ensure_axon_hooks_shim()

    x = np.ascontiguousarray(np.asarray(x, dtype=np.float32))
    weight = np.ascontiguousarray(np.asarray(weight, dtype=np.float32))
    counts = np.asarray(counts).astype(np.int64)
    assert counts.shape == (G,)

    mt, in_maps, metas = _prepare(x, weight, counts)
    if sum(mt) == 0:
        return np.zeros((N_TOK, D_OUT), np.float32), None
    if mt not in _cache:
        _cache[mt] = _build(mt)
    nc = _cache[mt]

    res = run_bass_kernel_spmd(
        nc,
        in_maps,
        core_ids=list(range(M_CORES)),
        trace=trace,
        trace_cores=trace_cores,
    )
    out = np.zeros((N_TOK, D_OUT), np.float32)
    for c in range(M_CORES):
        yc = np.asarray(res.results[c]["y"], dtype=np.float32)  # [NQ, P, n_mtiles, NQW]
        n_mtiles = yc.shape[2]
        # -> [n_mtiles, P, NQ, NQW] -> [n_mtiles*P, D_OUT]
        yc = yc.transpose(2, 1, 0, 3).reshape(n_mtiles * P, D_OUT)
        for mi0, s, n in metas[c]:
            if n > 0:
                out[s : s + n] = yc[mi0 * P : mi0 * P + n]
    return out, res


def kernel(x, weight, num_inputs_per_group):
    out, _ = _run(x, weight, num_inputs_per_group)
    return out

